# revision 1
# baseline (speedup 1.0000x reference)
"""Co-attention head kernel for 8 Trainium2 NeuronCores.

Reference computation (H=4096, heads=4, d=1024, N=1024):
    q/k/v[h] = node1|node2 @ W{q,k,v}[h] + b        ([N, d] per head)
    r[h]     = (q[h] @ k[h]^T * 1/sqrt(d)) .* v[h]  (elementwise, N==d)
    out      = LayerNorm(concat_h r[h])             ([N, 4096])

Sharding: 8 cores = 4 heads x 2 halves. Core c=(h=c//2, s=c%2):
  - computes qT/kT = W^T @ actT for its 512-wide f-slice of head h
    (weights pre-sliced+scaled on host; activations pre-transposed on host)
  - partial scores[n, m] over its f-half -> pair ReduceScatter(add)
    gives it rows n-own (even core: 0:512, odd: 512:1024) of full scores
  - v[n-own, :] natural via stationary n2T[:, n-own] blocks (host-sliced
    "n2v") and moving full Wv[h]
  - r = scores .* v; LayerNorm stats via AllReduce over same-parity cores
  - writes out block [512, 1024]; host assembles the [1024, 4096] output.

All matmuls run as float32r (full PE rate at free-dim>=256, ~2e-4 rel err).
NB: tensor_tensor_reduce crashes TRN2 hw via this toolchain — use plain
tensor_mul + tensor_reduce instead.
"""

from contextlib import ExitStack

import numpy as np

import concourse.bass as bass
import concourse.tile as tile
from concourse import bacc, mybir
from concourse.bass_utils import run_bass_kernel_spmd

F32 = mybir.dt.float32
F32R = mybir.dt.float32r

H_DIM = 4096
N_HEADS = 4
D_HEAD = 1024
N = 1024
LN_EPS = 1e-5
N_CORES = 8
SCALE = 1.0 / 32.0  # 1/sqrt(D_HEAD)

K_TILES = H_DIM // 128  # 32


WIRE = "f32r"

ALU = mybir.AluOpType
ACT_FN = mybir.ActivationFunctionType


def _bcast_ap(ap: bass.AP, parts: int = 128) -> bass.AP:
    """[n] DRAM vector viewed as [parts, n] with 0-stride partitions."""
    return bass.AP(tensor=ap.tensor, offset=ap.offset, ap=[[0, parts], *ap.ap])


def build_program(no_collectives: bool = False, reps: int = 1, wire: str = WIRE):
    WD = F32R if wire == "f32r" else mybir.dt.bfloat16
    KB = 2 if wire == "f32r" else 4  # k-tiles per stream DMA (SBUF-limited for f32)
    nc = bacc.Bacc("TRN2", target_bir_lowering=False, debug=False, num_devices=N_CORES)

    n1t = nc.dram_tensor("n1t", [H_DIM, N], WD, kind="ExternalInput").ap()
    n2t = nc.dram_tensor("n2t", [H_DIM, N], WD, kind="ExternalInput").ap()
    n2v = nc.dram_tensor("n2v", [H_DIM, 512], WD, kind="ExternalInput").ap()
    wq = nc.dram_tensor("wq", [H_DIM, 512], WD, kind="ExternalInput").ap()
    wk = nc.dram_tensor("wk", [H_DIM, 512], WD, kind="ExternalInput").ap()
    wv = nc.dram_tensor("wv", [H_DIM, D_HEAD], WD, kind="ExternalInput").ap()
    bq = nc.dram_tensor("bq", [512], F32, kind="ExternalInput").ap()
    bk = nc.dram_tensor("bk", [512], F32, kind="ExternalInput").ap()
    bv = nc.dram_tensor("bv", [D_HEAD], F32, kind="ExternalInput").ap()
    gam = nc.dram_tensor("gam", [D_HEAD], F32, kind="ExternalInput").ap()
    bet = nc.dram_tensor("bet", [D_HEAD], F32, kind="ExternalInput").ap()
    out = nc.dram_tensor("out", [512, N], F32, kind="ExternalOutput").ap()

    # paired k-tile views: [128, K_TILES, free]; col-group a = DRAM rows a*128+p
    n1t_2 = n1t.rearrange("(a p) n -> p a n", p=128)
    n2t_2 = n2t.rearrange("(a p) n -> p a n", p=128)
    n2v_2 = n2v.rearrange("(a p) n -> p a n", p=128)
    wq_2 = wq.rearrange("(a p) f -> p a f", p=128)
    wk_2 = wk.rearrange("(a p) f -> p a f", p=128)
    wv_2 = wv.rearrange("(a p) f -> p a f", p=128)

    with tile.TileContext(nc) as tc, ExitStack() as ctx:
        singles = ctx.enter_context(tc.tile_pool(name="singles", bufs=1))
        streams = ctx.enter_context(tc.tile_pool(name="streams", bufs=3))
        resident = ctx.enter_context(tc.tile_pool(name="resident", bufs=1))
        ps = ctx.enter_context(tc.tile_pool(name="ps", bufs=1, space="PSUM"))
        sc_pool = ctx.enter_context(tc.tile_pool(name="sc", bufs=2))
        fin = ctx.enter_context(tc.tile_pool(name="fin", bufs=1))
        dram = ctx.enter_context(tc.tile_pool(name="dram", bufs=1, space="DRAM"))

        # ---- constants (loaded once) ----
        bq_sb = singles.tile([128, 4], F32)
        nc.sync.dma_start(out=bq_sb, in_=bq.rearrange("(b p) -> p b", p=128))
        bk_sb = singles.tile([128, 4], F32)
        nc.sync.dma_start(out=bk_sb, in_=bk.rearrange("(b p) -> p b", p=128))
        bv_b = singles.tile([128, D_HEAD], F32)
        nc.sync.dma_start(out=bv_b, in_=_bcast_ap(bv))
        gam_b = singles.tile([128, D_HEAD], F32)
        nc.sync.dma_start(out=gam_b, in_=_bcast_ap(gam))
        bet_b = singles.tile([128, D_HEAD], F32)
        nc.sync.dma_start(out=bet_b, in_=_bcast_ap(bet))
        eps_sb = singles.tile([128, 1], F32)
        nc.vector.memset(eps_sb, LN_EPS)

        def emit_rep():
            rs_in = dram.tile([N, N], F32, name="rs_in", tag="rs_in", bufs=2)
            rs_out = dram.tile([512, N], F32, name="rs_out", tag="rs_out", bufs=2)
            ar_in = dram.tile([512, 2], F32, name="ar_in", tag="ar_in", bufs=2)
            ar_out = dram.tile([512, 2], F32, name="ar_out", tag="ar_out", bufs=2)
            # resident projection outputs (tags shared across reps)
            qT = [
                resident.tile([128, N], WD, name=f"qT{f}", tag=f"qT{f}")
                for f in range(4)
            ]
            kT = [
                resident.tile([128, N], WD, name=f"kT{f}", tag=f"kT{f}")
                for f in range(4)
            ]
            v_sb = [
                resident.tile([128, D_HEAD], F32, name=f"v{t}", tag=f"v{t}", bufs=2)
                for t in range(4)
            ]

            def projection(act2, w2, bias_fn):
                psums = [
                    [
                        ps.tile([128, 512], F32, name=f"pp{f}_{j}", tag=f"pp{f}_{j}")
                        for j in range(2)
                    ]
                    for f in range(4)
                ]
                for k4 in range(K_TILES // KB):
                    a_t = streams.tile([128, KB, N], WD, name="a_t", tag="a_t")
                    nc.sync.dma_start(out=a_t, in_=act2[:, KB * k4 : KB * k4 + KB, :])
                    w_t = streams.tile([128, KB, 512], WD, name="w_t", tag="w_t")
                    nc.scalar.dma_start(out=w_t, in_=w2[:, KB * k4 : KB * k4 + KB, :])
                    for a in range(KB):
                        first = k4 == 0 and a == 0
                        last = k4 == K_TILES // KB - 1 and a == KB - 1
                        for f in range(4):
                            for j in range(2):
                                nc.tensor.matmul(
                                    psums[f][j][:],
                                    w_t[:, a, f * 128 : (f + 1) * 128],
                                    a_t[:, a, j * 512 : (j + 1) * 512],
                                    start=first,
                                    stop=last,
                                )
                for f in range(4):
                    for j in range(2):
                        bias_fn(f, j, psums[f][j])

            # ---- Q phase ----
            projection(
                n1t_2,
                wq_2,
                lambda f, j, p: nc.vector.tensor_scalar(
                    out=qT[f][:, j * 512 : (j + 1) * 512],
                    in0=p[:],
                    scalar1=bq_sb[:, f : f + 1],
                    scalar2=None,
                    op0=ALU.add,
                ),
            )
            # ---- K phase ----
            projection(
                n2t_2,
                wk_2,
                lambda f, j, p: nc.vector.tensor_scalar(
                    out=kT[f][:, j * 512 : (j + 1) * 512],
                    in0=p[:],
                    scalar1=bk_sb[:, f : f + 1],
                    scalar2=None,
                    op0=ALU.add,
                ),
            )

            # ---- scores phase (partial over local f-half) + pair RS ----
            for nb in range(8):
                sc_sb = sc_pool.tile([128, N], F32, name="sc_sb", tag="sc_sb")
                for mh in range(2):
                    sc_ps = ps.tile(
                        [128, 512], F32, name=f"sc_ps{nb}_{mh}", tag=f"pp{nb % 4}_{mh}"
                    )
                    for ft in range(4):
                        nc.tensor.matmul(
                            sc_ps[:],
                            qT[ft][:, nb * 128 : (nb + 1) * 128],
                            kT[ft][:, mh * 512 : (mh + 1) * 512],
                            start=(ft == 0),
                            stop=(ft == 3),
                        )
                    nc.vector.tensor_copy(
                        out=sc_sb[:, mh * 512 : (mh + 1) * 512], in_=sc_ps[:]
                    )
                nc.gpsimd.dma_start(out=rs_in[nb * 128 : (nb + 1) * 128, :], in_=sc_sb)
            if no_collectives:
                nc.sync.dma_start(out=rs_out[:], in_=rs_in[0:512, :])
            else:
                nc.gpsimd.collective_compute(
                    "ReduceScatter",
                    ALU.add,
                    replica_groups=[[0, 1], [2, 3], [4, 5], [6, 7]],
                    ins=[rs_in[:].opt()],
                    outs=[rs_out[:].opt()],
                )

            # ---- V phase: stationary n2v blocks, moving full wv ----
            vps = [
                [
                    ps.tile([128, 512], F32, name=f"vp{t}_{j}", tag=f"pp{t}_{j}")
                    for j in range(2)
                ]
                for t in range(4)
            ]
            for k4 in range(K_TILES // KB):
                nv_t = streams.tile([128, KB, 512], WD, name="nv_t", tag="nv_t")
                nc.scalar.dma_start(out=nv_t, in_=n2v_2[:, KB * k4 : KB * k4 + KB, :])
                wv_t = streams.tile([128, KB, D_HEAD], WD, name="wv_t", tag="wv_t")
                nc.scalar.dma_start(out=wv_t, in_=wv_2[:, KB * k4 : KB * k4 + KB, :])
                for a in range(KB):
                    first = k4 == 0 and a == 0
                    last = k4 == K_TILES // KB - 1 and a == KB - 1
                    for t in range(4):
                        for j in range(2):
                            nc.tensor.matmul(
                                vps[t][j][:],
                                nv_t[:, a, t * 128 : (t + 1) * 128],
                                wv_t[:, a, j * 512 : (j + 1) * 512],
                                start=first,
                                stop=last,
                            )
            for t in range(4):
                for j in range(2):
                    nc.vector.tensor_add(
                        out=v_sb[t][:, j * 512 : (j + 1) * 512],
                        in0=vps[t][j][:],
                        in1=bv_b[:, j * 512 : (j + 1) * 512],
                    )

            # ---- final: r = sc .* v, LN stats, quad AR, normalize ----
            st_all = fin.tile([128, 4, 2], F32, name="st_all", tag="st_all", bufs=2)
            r_tiles = []
            for t in range(4):
                scr_t = fin.tile([128, N], F32, name=f"scr{t}", tag="scr", bufs=2)
                nc.gpsimd.dma_start(out=scr_t, in_=rs_out[t * 128 : (t + 1) * 128, :])
                sc_t = scr_t[:]
                r_t = fin.tile([128, N], F32, name=f"r{t}", tag=f"r{t}")
                nc.vector.tensor_mul(out=r_t[:], in0=sc_t, in1=v_sb[t][:])
                nc.vector.tensor_reduce(
                    out=st_all[:, t, 0:1], in_=r_t[:], axis=mybir.AxisListType.X, op=ALU.add
                )
                sq_t = fin.tile([128, N], F32, name="sq_t", tag="sq_t", bufs=1)
                nc.vector.tensor_mul(out=sq_t[:], in0=r_t[:], in1=r_t[:])
                nc.vector.tensor_reduce(
                    out=st_all[:, t, 1:2], in_=sq_t[:], axis=mybir.AxisListType.X, op=ALU.add
                )
                r_tiles.append(r_t)
            ar_in_2 = ar_in[:].rearrange("(b p) c -> p b c", p=128)
            ar_out_2 = ar_out[:].rearrange("(b p) c -> p b c", p=128)
            nc.gpsimd.dma_start(out=ar_in_2, in_=st_all)
            if no_collectives:
                nc.sync.dma_start(out=ar_out[:], in_=ar_in[:])
            else:
                nc.gpsimd.collective_compute(
                    "AllReduce",
                    ALU.add,
                    replica_groups=[[0, 2, 4, 6], [1, 3, 5, 7]],
                    ins=[ar_in[:].opt()],
                    outs=[ar_out[:].opt()],
                )
            tot_all = fin.tile([128, 4, 2], F32, name="tot_all", tag="tot_all", bufs=2)
            nc.gpsimd.dma_start(out=tot_all, in_=ar_out_2)
            inv_h = 1.0 / float(H_DIM)
            for t in range(4):
                mu_t = fin.tile([128, 1], F32, name=f"mu{t}", tag=f"mu{t}")
                nc.vector.tensor_scalar_mul(out=mu_t, in0=tot_all[:, t, 0:1], scalar1=inv_h)
                msq_t = fin.tile([128, 1], F32, name=f"msq{t}", tag=f"msq{t}")
                nc.vector.tensor_mul(out=msq_t, in0=mu_t, in1=mu_t)
                var_t = fin.tile([128, 1], F32, name=f"var{t}", tag=f"var{t}")
                nc.vector.tensor_scalar(
                    out=var_t,
                    in0=tot_all[:, t, 1:2],
                    scalar1=inv_h,
                    scalar2=msq_t[:, 0:1],
                    op0=ALU.mult,
                    op1=ALU.subtract,
                )
                nc.scalar.activation(
                    out=var_t, in_=var_t, func=ACT_FN.Sqrt, bias=eps_sb[:], scale=1.0
                )
                nc.vector.reciprocal(out=var_t, in_=var_t)
                o_t = fin.tile([128, N], F32, name="o_t", tag="o_t", bufs=2)[:]
                nc.vector.tensor_scalar(
                    out=o_t,
                    in0=r_tiles[t][:],
                    scalar1=mu_t[:, 0:1],
                    scalar2=var_t[:, 0:1],
                    op0=ALU.subtract,
                    op1=ALU.mult,
                )
                nc.vector.tensor_mul(out=o_t, in0=o_t, in1=gam_b[:])
                nc.vector.tensor_add(out=o_t, in0=o_t, in1=bet_b[:])
                nc.sync.dma_start(out=out[t * 128 : (t + 1) * 128, :], in_=o_t)

        for _ in range(reps):
            emit_rep()

    nc.compile()
    return nc


_NC = None


def _get_program():
    global _NC
    if _NC is None:
        _NC = build_program()
    return _NC


def make_in_maps(node1, node2, Wq, bq, Wk, bk, Wv, bv, gamma, beta, wire: str = WIRE):
    import ml_dtypes
    f32 = np.float32
    wd = np.float32 if wire == "f32r" else ml_dtypes.bfloat16
    n1t = np.ascontiguousarray(np.asarray(node1).T, dtype=f32)
    n2t = np.ascontiguousarray(np.asarray(node2).T, dtype=f32)
    n1t_w = n1t.astype(wd)
    n2t_w = n2t.astype(wd)
    in_maps = []
    for c in range(N_CORES):
        h, s = c // 2, c % 2
        fsl = slice(s * 512, (s + 1) * 512)
        in_maps.append(
            {
                "n1t": n1t_w,
                "n2t": n2t_w,
                "n2v": np.ascontiguousarray(n2t_w[:, fsl]),
                "wq": np.ascontiguousarray(Wq[h][:, fsl] * SCALE).astype(wd),
                "wk": np.ascontiguousarray(Wk[h][:, fsl]).astype(wd),
                "wv": np.ascontiguousarray(Wv[h]).astype(wd),
                "bq": np.ascontiguousarray(bq[h][fsl] * SCALE, dtype=f32),
                "bk": np.ascontiguousarray(bk[h][fsl], dtype=f32),
                "bv": np.ascontiguousarray(bv[h], dtype=f32),
                "gam": np.ascontiguousarray(gamma[h * 1024 : (h + 1) * 1024], dtype=f32),
                "bet": np.ascontiguousarray(beta[h * 1024 : (h + 1) * 1024], dtype=f32),
            }
        )
    return in_maps


def assemble(results):
    out = np.empty((N, H_DIM), np.float32)
    for c in range(N_CORES):
        h, s = c // 2, c % 2
        out[s * 512 : (s + 1) * 512, h * 1024 : (h + 1) * 1024] = results[c]["out"]
    return out


def kernel(node1, node2, Wq, bq, Wk, bk, Wv, bv, gamma, beta):
    nc = _get_program()
    in_maps = make_in_maps(node1, node2, Wq, bq, Wk, bk, Wv, bv, gamma, beta)
    res = run_bass_kernel_spmd(nc, in_maps, list(range(N_CORES)))
    return assemble(res.results)



# revision 3
# speedup vs baseline: 1.6474x; 1.6474x over previous
"""Co-attention head kernel for 8 Trainium2 NeuronCores — v2 (n-sharded scores).

Reference computation (H=4096, heads=4, d=1024, N=1024):
    q/k/v[h] = node1|node2 @ W{q,k,v}[h] + b        ([N, d] per head)
    r[h]     = (q[h] @ k[h]^T * 1/sqrt(d)) .* v[h]  (elementwise, N==d)
    out      = LayerNorm(concat_h r[h])             ([N, 4096])

Sharding: 8 cores = 4 heads x 2 node-halves. Core c=(h=c//2, s=c%2) owns
rows n-own = s*512:(s+1)*512 of q/scores/v for head h:
  - kT[f, m-own]  = Wk[h]^T @ n2T[:, m-own]   (full f=1024, own m-half)
    -> pair AllGather of kT halves (1 MB bf16) gives full kT [f, 1024]
  - qT[f, n-own]  = Wq[h]^T @ n1T[:, n-own]   (scaled by 1/32 on host)
  - scores[n-own, :] = qT^T @ kT              (full contraction, no RS)
  - v[n-own, :]   = n2T[:, n-own]^T @ Wv[h]   (n2T half reused from K phase)
  - r = scores .* v; LayerNorm stats via AllReduce over same-parity cores
  - writes out block [512, 1024]; host assembles the [1024, 4096] output.

All wire data is bf16 (DMA traffic ~36 MB/core/rep vs 72 MB for f32);
PSUM accumulation is fp32. NB: tensor_tensor_reduce crashes TRN2 hw via
this toolchain — use plain tensor_mul + tensor_reduce instead.
"""

from contextlib import ExitStack

import numpy as np

import concourse.bass as bass
import concourse.tile as tile
from concourse import bacc, mybir
from concourse.bass_utils import run_bass_kernel_spmd

F32 = mybir.dt.float32
BF16 = mybir.dt.bfloat16

H_DIM = 4096
N_HEADS = 4
D_HEAD = 1024
N = 1024
LN_EPS = 1e-5
N_CORES = 8
SCALE = 1.0 / 32.0  # 1/sqrt(D_HEAD)

K_TILES = H_DIM // 128  # 32
KB = 4  # k-tiles per stream DMA
NSTEP = K_TILES // KB  # 8

ALU = mybir.AluOpType
ACT_FN = mybir.ActivationFunctionType


def _bcast_ap(ap: bass.AP, parts: int = 128) -> bass.AP:
    """[n] DRAM vector viewed as [parts, n] with 0-stride partitions."""
    return bass.AP(tensor=ap.tensor, offset=ap.offset, ap=[[0, parts], *ap.ap])


def build_program(no_collectives: bool = False, reps: int = 1, wire: str = "bf16"):
    nc = bacc.Bacc("TRN2", target_bir_lowering=False, debug=False, num_devices=N_CORES)

    n1o = nc.dram_tensor("n1o", [H_DIM, 512], BF16, kind="ExternalInput").ap()
    n2o = nc.dram_tensor("n2o", [H_DIM, 512], BF16, kind="ExternalInput").ap()
    wq = nc.dram_tensor("wq", [H_DIM, D_HEAD], BF16, kind="ExternalInput").ap()
    wk = nc.dram_tensor("wk", [H_DIM, D_HEAD], BF16, kind="ExternalInput").ap()
    wv = nc.dram_tensor("wv", [H_DIM, D_HEAD], BF16, kind="ExternalInput").ap()
    bq = nc.dram_tensor("bq", [D_HEAD], F32, kind="ExternalInput").ap()
    bk = nc.dram_tensor("bk", [D_HEAD], F32, kind="ExternalInput").ap()
    bv = nc.dram_tensor("bv", [D_HEAD], F32, kind="ExternalInput").ap()
    gam = nc.dram_tensor("gam", [D_HEAD], F32, kind="ExternalInput").ap()
    bet = nc.dram_tensor("bet", [D_HEAD], F32, kind="ExternalInput").ap()
    out = nc.dram_tensor("out", [512, D_HEAD], F32, kind="ExternalOutput").ap()

    # k-tile views: [128, K_TILES, cols]; col-group a = DRAM rows a*128+p
    n1o_2 = n1o.rearrange("(a p) n -> p a n", p=128)
    n2o_2 = n2o.rearrange("(a p) n -> p a n", p=128)
    wq_2 = wq.rearrange("(a p) f -> p a f", p=128)
    wk_2 = wk.rearrange("(a p) f -> p a f", p=128)
    wv_2 = wv.rearrange("(a p) f -> p a f", p=128)

    with tile.TileContext(nc) as tc, ExitStack() as ctx:
        singles = ctx.enter_context(tc.tile_pool(name="singles", bufs=1))
        streams = ctx.enter_context(tc.tile_pool(name="streams", bufs=3))
        resident = ctx.enter_context(tc.tile_pool(name="resident", bufs=1))
        ps = ctx.enter_context(tc.tile_pool(name="ps", bufs=1, space="PSUM"))
        fin = ctx.enter_context(tc.tile_pool(name="fin", bufs=1))
        dram = ctx.enter_context(tc.tile_pool(name="dram", bufs=1, space="DRAM"))

        # ---- constants (loaded once) ----
        bq_sb = singles.tile([128, 8], F32)
        nc.sync.dma_start(out=bq_sb, in_=bq.rearrange("(b p) -> p b", p=128))
        bk_sb = singles.tile([128, 8], F32)
        nc.sync.dma_start(out=bk_sb, in_=bk.rearrange("(b p) -> p b", p=128))
        bv_b = singles.tile([128, D_HEAD], F32)
        nc.sync.dma_start(out=bv_b, in_=_bcast_ap(bv))
        gam_b = singles.tile([128, D_HEAD], F32)
        nc.sync.dma_start(out=gam_b, in_=_bcast_ap(gam))
        bet_b = singles.tile([128, D_HEAD], F32)
        nc.sync.dma_start(out=bet_b, in_=_bcast_ap(bet))
        eps_sb = singles.tile([128, 1], F32)
        nc.vector.memset(eps_sb, LN_EPS)

        def emit_rep():
            ag_in = dram.tile([D_HEAD, 512], BF16, name="ag_in", tag="ag_in", bufs=2)
            ag_out = dram.tile(
                [2 * D_HEAD, 512], BF16, name="ag_out", tag="ag_out", bufs=2
            )
            ar_in = dram.tile([512, 2], F32, name="ar_in", tag="ar_in", bufs=2)
            ar_out = dram.tile([512, 2], F32, name="ar_out", tag="ar_out", bufs=2)

            # resident tiles (tags shared across reps; bufs=2 => next rep overlaps)
            n2o_sb = resident.tile(
                [128, K_TILES, 512], BF16, name="n2o_sb", tag="n2o_sb", bufs=1
            )
            qT = [
                resident.tile([128, 512], BF16, name=f"qT{f}", tag=f"qT{f}")
                for f in range(8)
            ]
            kT = [
                resident.tile([128, D_HEAD], BF16, name=f"kT{f}", tag=f"kT{f}")
                for f in range(8)
            ]
            kTo = [
                resident.tile([128, 512], BF16, name=f"kTo{f}", tag=f"kTo{f}")
                for f in range(8)
            ]
            v_sb = [
                resident.tile([128, D_HEAD], F32, name=f"v{t}", tag=f"v{t}", bufs=2)
                for t in range(4)
            ]

            # n2T own half: loaded once, reused by K (moving) and V (stationary)
            for c4 in range(NSTEP):
                nc.sync.dma_start(
                    out=n2o_sb[:, KB * c4 : KB * c4 + KB, :],
                    in_=n2o_2[:, KB * c4 : KB * c4 + KB, :],
                )

            # ---- K projection: kT_own[f, m-own] = wk^T @ n2o ----
            kps = [
                ps.tile([128, 512], F32, name=f"pp{f}", tag=f"pp{f}")
                for f in range(8)
            ]
            for k4 in range(NSTEP):
                wk_t = streams.tile([128, KB, D_HEAD], BF16, name="wk_t", tag="w_t")
                nc.scalar.dma_start(out=wk_t, in_=wk_2[:, KB * k4 : KB * k4 + KB, :])
                for a in range(KB):
                    first = k4 == 0 and a == 0
                    last = k4 == NSTEP - 1 and a == KB - 1
                    for f in range(8):
                        nc.tensor.matmul(
                            kps[f][:],
                            wk_t[:, a, f * 128 : (f + 1) * 128],
                            n2o_sb[:, k4 * KB + a, :],
                            start=first,
                            stop=last,
                        )
            for f in range(8):
                nc.vector.tensor_scalar(
                    out=kTo[f][:],
                    in0=kps[f][:],
                    scalar1=bk_sb[:, f : f + 1],
                    scalar2=None,
                    op0=ALU.add,
                )
                nc.gpsimd.dma_start(
                    out=ag_in[f * 128 : (f + 1) * 128, :], in_=kTo[f][:]
                )

            # pair AllGather of kT halves (even rank = m 0:512, odd = 512:1024)
            if no_collectives:
                nc.gpsimd.dma_start(out=ag_out[0:D_HEAD, :], in_=ag_in[:])
                nc.gpsimd.dma_start(out=ag_out[D_HEAD : 2 * D_HEAD, :], in_=ag_in[:])
            else:
                nc.gpsimd.collective_compute(
                    "AllGather",
                    ALU.bypass,
                    replica_groups=[[0, 1], [2, 3], [4, 5], [6, 7]],
                    ins=[ag_in[:].opt()],
                    outs=[ag_out[:].opt()],
                )

            # ---- Q projection: qT[f, n-own] = (wq*scale)^T @ n1o ----
            qps = [
                ps.tile([128, 512], F32, name=f"qp{f}", tag=f"pp{f}")
                for f in range(8)
            ]
            for k4 in range(NSTEP):
                a_t = streams.tile([128, KB, 512], BF16, name="a_t", tag="a_t")
                nc.sync.dma_start(out=a_t, in_=n1o_2[:, KB * k4 : KB * k4 + KB, :])
                wq_t = streams.tile([128, KB, D_HEAD], BF16, name="wq_t", tag="w_t")
                nc.scalar.dma_start(out=wq_t, in_=wq_2[:, KB * k4 : KB * k4 + KB, :])
                for a in range(KB):
                    first = k4 == 0 and a == 0
                    last = k4 == NSTEP - 1 and a == KB - 1
                    for f in range(8):
                        nc.tensor.matmul(
                            qps[f][:],
                            wq_t[:, a, f * 128 : (f + 1) * 128],
                            a_t[:, a, :],
                            start=first,
                            stop=last,
                        )
            for f in range(8):
                nc.vector.tensor_scalar(
                    out=qT[f][:],
                    in0=qps[f][:],
                    scalar1=bq_sb[:, f : f + 1],
                    scalar2=None,
                    op0=ALU.add,
                )

            # assemble full kT from AllGather output (uniform layout per rank)
            for f in range(8):
                nc.gpsimd.dma_start(
                    out=kT[f][:, 0:512], in_=ag_out[f * 128 : (f + 1) * 128, :]
                )
                nc.gpsimd.dma_start(
                    out=kT[f][:, 512:1024],
                    in_=ag_out[D_HEAD + f * 128 : D_HEAD + (f + 1) * 128, :],
                )

            # ---- scores[n-own, :] = qT^T @ kT (full f contraction) ----
            sc_tiles = []
            for nb in range(4):
                sc_sb = fin.tile([128, N], F32, name=f"sc{nb}", tag=f"sc{nb}")
                for mh in range(2):
                    sc_ps = ps.tile(
                        [128, 512], F32, name=f"sc_ps{nb}_{mh}", tag=f"pp{2 * nb + mh}"
                    )
                    for ft in range(8):
                        nc.tensor.matmul(
                            sc_ps[:],
                            qT[ft][:, nb * 128 : (nb + 1) * 128],
                            kT[ft][:, mh * 512 : (mh + 1) * 512],
                            start=(ft == 0),
                            stop=(ft == 7),
                        )
                    nc.vector.tensor_copy(
                        out=sc_sb[:, mh * 512 : (mh + 1) * 512], in_=sc_ps[:]
                    )
                sc_tiles.append(sc_sb)

            # ---- V: v[n-own, :] = n2o^T @ wv (stationary n2o blocks) ----
            vps = [
                [
                    ps.tile([128, 512], F32, name=f"vp{t}_{j}", tag=f"pp{2 * t + j}")
                    for j in range(2)
                ]
                for t in range(4)
            ]
            for k4 in range(NSTEP):
                wv_t = streams.tile([128, KB, D_HEAD], BF16, name="wv_t", tag="w_t")
                nc.scalar.dma_start(out=wv_t, in_=wv_2[:, KB * k4 : KB * k4 + KB, :])
                for a in range(KB):
                    first = k4 == 0 and a == 0
                    last = k4 == NSTEP - 1 and a == KB - 1
                    for t in range(4):
                        for j in range(2):
                            nc.tensor.matmul(
                                vps[t][j][:],
                                n2o_sb[:, k4 * KB + a, t * 128 : (t + 1) * 128],
                                wv_t[:, a, j * 512 : (j + 1) * 512],
                                start=first,
                                stop=last,
                            )
            for t in range(4):
                for j in range(2):
                    nc.vector.tensor_add(
                        out=v_sb[t][:, j * 512 : (j + 1) * 512],
                        in0=vps[t][j][:],
                        in1=bv_b[:, j * 512 : (j + 1) * 512],
                    )

            # ---- final: r = sc .* v, LN stats, quad AR, normalize ----
            st_all = fin.tile([128, 4, 2], F32, name="st_all", tag="st_all", bufs=2)
            r_tiles = []
            for t in range(4):
                r_t = fin.tile([128, N], F32, name=f"r{t}", tag=f"r{t}")
                nc.vector.tensor_mul(out=r_t[:], in0=sc_tiles[t][:], in1=v_sb[t][:])
                nc.vector.tensor_reduce(
                    out=st_all[:, t, 0:1], in_=r_t[:], axis=mybir.AxisListType.X, op=ALU.add
                )
                sq_t = fin.tile([128, N], F32, name="sq_t", tag="sq_t", bufs=1)
                nc.vector.tensor_mul(out=sq_t[:], in0=r_t[:], in1=r_t[:])
                nc.vector.tensor_reduce(
                    out=st_all[:, t, 1:2], in_=sq_t[:], axis=mybir.AxisListType.X, op=ALU.add
                )
                r_tiles.append(r_t)
            ar_in_2 = ar_in[:].rearrange("(b p) c -> p b c", p=128)
            ar_out_2 = ar_out[:].rearrange("(b p) c -> p b c", p=128)
            nc.gpsimd.dma_start(out=ar_in_2, in_=st_all)
            if no_collectives:
                nc.sync.dma_start(out=ar_out[:], in_=ar_in[:])
            else:
                nc.gpsimd.collective_compute(
                    "AllReduce",
                    ALU.add,
                    replica_groups=[[0, 2, 4, 6], [1, 3, 5, 7]],
                    ins=[ar_in[:].opt()],
                    outs=[ar_out[:].opt()],
                )
            tot_all = fin.tile([128, 4, 2], F32, name="tot_all", tag="tot_all", bufs=2)
            nc.gpsimd.dma_start(out=tot_all, in_=ar_out_2)
            inv_h = 1.0 / float(H_DIM)
            for t in range(4):
                mu_t = fin.tile([128, 1], F32, name=f"mu{t}", tag=f"mu{t}")
                nc.vector.tensor_scalar_mul(out=mu_t, in0=tot_all[:, t, 0:1], scalar1=inv_h)
                msq_t = fin.tile([128, 1], F32, name=f"msq{t}", tag=f"msq{t}")
                nc.vector.tensor_mul(out=msq_t, in0=mu_t, in1=mu_t)
                var_t = fin.tile([128, 1], F32, name=f"var{t}", tag=f"var{t}")
                nc.vector.tensor_scalar(
                    out=var_t,
                    in0=tot_all[:, t, 1:2],
                    scalar1=inv_h,
                    scalar2=msq_t[:, 0:1],
                    op0=ALU.mult,
                    op1=ALU.subtract,
                )
                nc.scalar.activation(
                    out=var_t, in_=var_t, func=ACT_FN.Sqrt, bias=eps_sb[:], scale=1.0
                )
                nc.vector.reciprocal(out=var_t, in_=var_t)
                o_t = fin.tile([128, N], F32, name="o_t", tag="o_t", bufs=2)[:]
                nc.vector.tensor_scalar(
                    out=o_t,
                    in0=r_tiles[t][:],
                    scalar1=mu_t[:, 0:1],
                    scalar2=var_t[:, 0:1],
                    op0=ALU.subtract,
                    op1=ALU.mult,
                )
                nc.vector.tensor_mul(out=o_t, in0=o_t, in1=gam_b[:])
                nc.vector.tensor_add(out=o_t, in0=o_t, in1=bet_b[:])
                nc.sync.dma_start(out=out[t * 128 : (t + 1) * 128, :], in_=o_t)

        for _ in range(reps):
            emit_rep()

    nc.compile()
    return nc


_NC = None


def _get_program():
    global _NC
    if _NC is None:
        _NC = build_program()
    return _NC


def make_in_maps(node1, node2, Wq, bq, Wk, bk, Wv, bv, gamma, beta, wire: str = "bf16"):
    import ml_dtypes

    f32 = np.float32
    wd = ml_dtypes.bfloat16
    n1t = np.ascontiguousarray(np.asarray(node1).T).astype(wd)
    n2t = np.ascontiguousarray(np.asarray(node2).T).astype(wd)
    in_maps = []
    for c in range(N_CORES):
        h, s = c // 2, c % 2
        nsl = slice(s * 512, (s + 1) * 512)
        in_maps.append(
            {
                "n1o": np.ascontiguousarray(n1t[:, nsl]),
                "n2o": np.ascontiguousarray(n2t[:, nsl]),
                "wq": np.ascontiguousarray(Wq[h] * SCALE).astype(wd),
                "wk": np.ascontiguousarray(Wk[h]).astype(wd),
                "wv": np.ascontiguousarray(Wv[h]).astype(wd),
                "bq": np.ascontiguousarray(bq[h] * SCALE, dtype=f32),
                "bk": np.ascontiguousarray(bk[h], dtype=f32),
                "bv": np.ascontiguousarray(bv[h], dtype=f32),
                "gam": np.ascontiguousarray(gamma[h * 1024 : (h + 1) * 1024], dtype=f32),
                "bet": np.ascontiguousarray(beta[h * 1024 : (h + 1) * 1024], dtype=f32),
            }
        )
    return in_maps


def assemble(results):
    out = np.empty((N, H_DIM), np.float32)
    for c in range(N_CORES):
        h, s = c // 2, c % 2
        out[s * 512 : (s + 1) * 512, h * 1024 : (h + 1) * 1024] = results[c]["out"]
    return out


def kernel(node1, node2, Wq, bq, Wk, bk, Wv, bv, gamma, beta):
    nc = _get_program()
    in_maps = make_in_maps(node1, node2, Wq, bq, Wk, bk, Wv, bv, gamma, beta)
    res = run_bass_kernel_spmd(nc, in_maps, list(range(N_CORES)))
    return assemble(res.results)
